# revision 64
# baseline (speedup 1.0000x reference)
"""Multi-head attention (projections + causal/padded softmax attention + output
projection + residual + LayerNorm) as a Bass/Tile kernel on 8 Trainium2 cores.

Sharding: tensor-parallel over heads within each batch. Core c handles batch
g = c // 4 and heads [4*(c%4), 4*(c%4)+4). Each core projects Q/K/V for its
4 heads over the full sequence, runs causal attention in a transposed layout
(scoresT[key, row]), and produces ctxT[dh, row]. One 8-way AllToAll per
head-pair redistributes ctxT with a fully STATIC slot map: slot j carries rows
[j*256, (j+1)*256) of the sender's batch, so core j ends up owning that row
range of BOTH batches (cores 0-3 receive batch-0 contributions from cores 0-3
and batch-1 contributions from cores 4-7 in distinct sender slots). No runtime
core-id addressing, no barriers: Tile orders staging DMAs before each
collective and the pair-0 collective overlaps pair-1's attention.

Layout trick: all matmul operands are pre-transposed/pre-cast on the host
(numpy) so every DMA is contiguous: qT/kT/vT = x^T as bf16, WqT/WkT/WvT/WoT =
W^T as bf16. The PE contracts over partitions, so the contraction dim (d_model
or d_head) always sits on the partition axis.

Softmax: scores are bounded (|s| ~ 5) so exp is computed without max
subtraction; both heads' scores share one 2-bank psum tile so a single
scalar-engine exp(scale*s + pad_bias) covers them, with the padding mask
folded into the per-key bias. The causal boundary adds a precomputed
triangular -1e9 bias onto the diagonal 128-col band (vector engine) before
exp. The denominator comes from augmenting V with a ones column (row dh of
ctxT psum = sum of probs); the divide uses a [1,R] fast-approx reciprocal
(the DVE RECIPROCAL op costs a flat ~3.3us) + partition broadcast + multiply.

PE p-state discipline: the TRN2 tensor engine runs at 1.2GHz until it has
been continuously busy ~3us (then 2.4GHz), so every stall halves throughput.
Attention is software-pipelined (ctx matmul of chunk kb-3 emitted between the
score matmuls of chunk kb), and all projection work that the first attention
row-range doesn't need (K slice 1, V chunks 4+, Q slices 1-3) is deferred and
interleaved into the first pair's attention stream as scalar-independent PE
filler. Weights load on the scalar DMA queue, P3 constants load after P1's
input stream, and a dummy warm-up collective absorbs the ~11us first-use
latency of the NRT collective stream.

PSUM budget (8 banks): sc=3x2 banks + ctx=2x1 = 8; the sc tag doubles as the
projection and Wo accumulators. A no-sync scheduler fence keeps the cc-gated
fetch DMAs from blocking the staging DMAs on the in-order sync queue.
"""

import math
from contextlib import ExitStack

import numpy as np
import ml_dtypes

import concourse.bass as bass
import concourse.mybir as mybir
import concourse.tile as tile
from concourse import bacc
from concourse.bass_utils import run_bass_kernel_spmd

BF16 = mybir.dt.bfloat16
F32 = mybir.dt.float32

NEG_INF = -1e9
LN_EPS = 1e-6


class Cfg:
    def __init__(self, B=2, S=2048, D=1024, H=16, dh=64, kmax=None):
        self.B, self.S, self.D, self.H, self.dh = B, S, D, H, dh
        # kmax: max(sen_len) — keys beyond are fully masked, so K/V
        # projection and the attention key loop stop at this bound.
        self.kmax = S if kmax is None else min(int(kmax), S)
        self.NC = 8                      # cores
        self.G = 4                       # cores per batch group
        self.HPC = H // self.G           # heads per core
        self.PAIRS = self.HPC // 2       # head pairs per core
        self.D4 = self.HPC * dh          # per-core projection width
        self.RQ = S // self.G            # rows per core in Wo/LN phase
        self.NR = 4                      # attention row ranges
        self.RNG = S // self.NR          # rows per range (== RQ)
        self.RSL = S // self.NC          # rows per A2A slot (256)
        self.DC = D // 128               # contraction chunks
        self.KCH = S // 128              # key chunks
        self.NS = max(1, S // 512)       # projection n-slices
        self.NSW = S // self.NS          # cols per n-slice
        self.WON = max(1, D // 512)      # Wo n-slices
        self.WONW = D // self.WON
        self.D4C = self.D4 // 128        # 128-chunks in per-core ctx width
        self.KB_MAX = -(-self.kmax // 128)          # key chunks actually used
        self.NS_K = -(-(self.KB_MAX * 128) // self.NSW)  # K-proj n-slices
        # Large kmax needs the SBUF for K/V state; drop the Q-interleave
        # buffers and run leaner pipelining in that case.
        self.LEAN = self.KB_MAX > 10
        assert self.RQ == self.RNG
        assert self.PAIRS >= 1 and self.HPC % 2 == 0


def build_program(cfg: Cfg):
    """Build the (SPMD-identical) Bass program."""
    nc = bacc.Bacc("TRN2", target_bir_lowering=False, debug=False,
                   num_devices=cfg.NC)

    S, D, dh = cfg.S, cfg.D, cfg.dh
    D4, RQ, RNG, RSL = cfg.D4, cfg.RQ, cfg.RNG, cfg.RSL

    # All inputs are pre-tiled on the host so every DMA is contiguous per
    # partition: x inputs as [ns, p, dc, cols], weights as [p, dc, outs].
    qT = nc.dram_tensor("qT", [cfg.NS, 128, cfg.DC, cfg.NSW], BF16,
                        kind="ExternalInput").ap()
    kT = nc.dram_tensor("kT", [cfg.NS_K, 128, cfg.DC, cfg.NSW], BF16,
                        kind="ExternalInput").ap()
    vT = nc.dram_tensor("vT", [cfg.KB_MAX, 128, cfg.DC, 128], BF16,
                        kind="ExternalInput").ap()
    wqT = nc.dram_tensor("wqT", [128, cfg.DC, D4], BF16,
                         kind="ExternalInput").ap()
    wkT = nc.dram_tensor("wkT", [128, cfg.DC, D4], BF16,
                         kind="ExternalInput").ap()
    wvT = nc.dram_tensor("wvT", [128, cfg.DC, D4], BF16,
                         kind="ExternalInput").ap()
    woT = nc.dram_tensor("woT", [128, cfg.DC, D], BF16,
                         kind="ExternalInput").ap()
    resid = nc.dram_tensor("resid", [128, cfg.G, D], F32,
                           kind="ExternalInput").ap()
    pad_bias = nc.dram_tensor("pad_bias", [128, cfg.KCH], F32,
                              kind="ExternalInput").ap()
    gamma = nc.dram_tensor("gamma", [1, D], BF16, kind="ExternalInput").ap()
    beta = nc.dram_tensor("beta", [1, D], BF16, kind="ExternalInput").ap()
    out_shard = nc.dram_tensor("out_shard", [RQ, D], BF16,
                               kind="ExternalOutput").ap()

    with tile.TileContext(nc) as tc, ExitStack() as ctx:
        consts = ctx.enter_context(tc.tile_pool(name="consts", bufs=1))
        xin = ctx.enter_context(tc.tile_pool(name="xin", bufs=2))
        proj = ctx.enter_context(tc.tile_pool(name="proj", bufs=1))
        att = ctx.enter_context(tc.tile_pool(name="att", bufs=2))
        small = ctx.enter_context(tc.tile_pool(name="small", bufs=2))
        lnp = ctx.enter_context(tc.tile_pool(name="lnp", bufs=2))
        ctxf = ctx.enter_context(tc.tile_pool(name="ctxf", bufs=1))
        dram = ctx.enter_context(
            tc.tile_pool(name="dram", bufs=1, space="DRAM"))
        psum = ctx.enter_context(
            tc.tile_pool(name="psum", bufs=1, space="PSUM"))

        # ---- prologue: all constants (incl. P3's, so P3 never waits) -------
        # Weights ride the scalar engine's DMA queue so the sync queue can
        # start streaming the K/Q/V activations immediately (parallel DMA).
        wq_sb = consts.tile([128, cfg.DC, D4], BF16)
        wk_sb = consts.tile([128, cfg.DC, D4], BF16)
        wv_sb = consts.tile([128, cfg.DC, D4], BF16)
        for w_sb, w_dram in ((wk_sb, wkT), (wv_sb, wvT), (wq_sb, wqT)):
            nc.scalar.dma_start(out=w_sb, in_=w_dram)

        pb_sb = consts.tile([128, cfg.KCH], F32)
        nc.scalar.dma_start(out=pb_sb, in_=pad_bias)

        # P3 constants (loaded after P1's input stream, see below)
        wo_sb = consts.tile([128, cfg.DC, D], BF16)
        g_row = consts.tile([1, D], BF16)
        b_row = consts.tile([1, D], BF16)
        gamma_bc = consts.tile([128, D], BF16)
        beta_bc = consts.tile([128, D], BF16)
        eps_sb = consts.tile([128, 1], F32)
        nc.vector.memset(eps_sb, LN_EPS)
        res_sb = consts.tile([128, cfg.G, D], F32)
        # causal triangle bias: tri[p, f] = 0 if f >= p else NEG_INF.
        # Added (by the vector engine) onto the diagonal 128-col band of the
        # scores before exp — keeps the gpsimd queue out of the PE's
        # dependency chain.
        tri = consts.tile([128, 128], F32)
        nc.vector.memset(tri, 0.0)
        nc.gpsimd.affine_select(
            out=tri, in_=tri, pattern=[[1, 128]], base=0,
            channel_multiplier=-1, compare_op=mybir.AluOpType.is_ge,
            fill=NEG_INF)

        # A2A buffers: one per head-pair; slot j = rows [j*RSL,(j+1)*RSL).
        a2a_in = [dram.tile([cfg.NC, 128, RSL], BF16, name=f"a2a_in{p}",
                            tag=f"a2a_in{p}") for p in range(cfg.PAIRS)]
        a2a_out = [dram.tile([cfg.NC, 128, RSL], BF16, name=f"a2a_out{p}",
                             tag=f"a2a_out{p}") for p in range(cfg.PAIRS)]

        # warm up the NRT collective stream during P1: the first collective
        # after the prelude barrier pays ~11us of trigger latency; a dummy
        # 4KB AllToAll absorbs it so cc0/cc1 start promptly.
        warm_in = dram.tile([cfg.NC, 128, 2], BF16, name="warm_in",
                            tag="warm_in")
        warm_out = dram.tile([cfg.NC, 128, 2], BF16, name="warm_out",
                             tag="warm_out")
        nc.gpsimd.collective_compute(
            "AllToAll", mybir.AluOpType.bypass,
            replica_groups=[list(range(cfg.NC))],
            ins=[warm_in[:]], outs=[warm_out[:]])

        # ---- P1: projections (K, V first so attention can start early) ----
        # K/V SBUF is sized to the kmax actually used, not full S.
        qhT_sb = proj.tile([128, cfg.PAIRS, S], BF16)
        khT_sb = proj.tile([128, cfg.PAIRS, cfg.NS_K * cfg.NSW], BF16)
        vh_sb = proj.tile([128, cfg.KB_MAX, cfg.HPC * (dh + 1)], BF16)

        def qk_proj(x_dram, w_sb, out_sb, ns_count=None, ns_start=0):
            for ns in range(ns_start,
                            ns_count if ns_count is not None else cfg.NS):
                x_ns = xin.tile([128, cfg.DC, cfg.NSW], BF16, tag="x_ns",
                                name="x_ns")
                nc.sync.dma_start(out=x_ns, in_=x_dram[ns])
                for pair in range(cfg.PAIRS):
                    ps = psum.tile([128, cfg.NSW], F32, tag="ctx", bufs=4,
                                   name="ps_pj")
                    for dc in range(cfg.DC):
                        nc.tensor.matmul(
                            ps, w_sb[:, dc, pair * 128:(pair + 1) * 128],
                            x_ns[:, dc, :],
                            start=dc == 0, stop=dc == cfg.DC - 1)
                    nc.vector.tensor_copy(
                        out=out_sb[:, pair, ns * cfg.NSW:(ns + 1) * cfg.NSW],
                        in_=ps)

        def v_chunk(kb):
            v_kb = xin.tile([128, cfg.DC, 128], BF16, tag="v_kb", bufs=3,
                            name="v_kb")
            nc.sync.dma_start(out=v_kb, in_=vT[kb])

            def mm(p, dc, psv):
                nc.tensor.matmul(psv, v_kb[:, dc, :], wv_sb[:, dc, :],
                                 start=dc == 0, stop=dc == cfg.DC - 1)
                if dc == cfg.DC - 1:
                    nc.vector.tensor_copy(
                        out=vh_sb[:, kb, :]
                        .rearrange("p (h e) -> p h e", e=dh + 1)[:, :, 0:dh],
                        in_=psv.rearrange("p (h e) -> p h e", e=dh))
                    nc.vector.memset(
                        vh_sb[:, kb, :]
                        .rearrange("p (h e) -> p h e", e=dh + 1)
                        [:, :, dh:dh + 1], 1.0)
            return [(None, dc, mm, D4) for dc in range(cfg.DC)]

        def xw_slice(ns, x_tile, w_sb, out_sb):
            def mm(p, dc, qp):
                nc.tensor.matmul(
                    qp, w_sb[:, dc, p * 128:(p + 1) * 128],
                    x_tile[:, dc, :], start=dc == 0, stop=dc == cfg.DC - 1)
                if dc == cfg.DC - 1:
                    nc.vector.tensor_copy(
                        out=out_sb[:, p, ns * cfg.NSW:(ns + 1) * cfg.NSW],
                        in_=qp)
            return [(p, dc, mm, cfg.NSW)
                    for p in range(cfg.PAIRS) for dc in range(cfg.DC)]

        class Job:
            """A deferred stream of projection matmuls used as PE filler
            inside the attention loop (keeps the p-state ramp alive)."""
            def __init__(self, items):
                self.items = items
                self.idx = 0
                self.ps = None

            def emit(self, n):
                for _ in range(n):
                    if self.idx >= len(self.items):
                        return
                    p, dc, mm, width = self.items[self.idx]
                    self.idx += 1
                    if dc == 0:
                        self.ps = psum.tile([128, width], F32, tag="ctx",
                                            bufs=4, name="fill_ps")
                    mm(p, dc, self.ps)

            def done(self):
                return self.idx >= len(self.items)

            def finish(self):
                self.emit(len(self.items) - self.idx)

        # P1 proper: K slice 0, V chunks 0..3, Q slice 0 — the minimum the
        # first attention row-range needs. The rest becomes filler.
        # The very first x transfer is split in half so the first matmul
        # starts ~2us sooner (it only waits on 512KB, not 1MB).
        x_k0 = xin.tile([128, cfg.DC, cfg.NSW], BF16, tag="x_ns",
                        name="x_k0")
        hw_ = cfg.NSW // 2
        nc.sync.dma_start(out=x_k0[:, :, 0:hw_], in_=kT[0][:, :, 0:hw_])
        nc.sync.dma_start(out=x_k0[:, :, hw_:], in_=kT[0][:, :, hw_:])
        for pair in range(cfg.PAIRS):
            for half in range(2):
                ps = psum.tile([128, cfg.NSW], F32, tag="ctx", bufs=4,
                               name="ps_k0")
                cs = slice(half * hw_, half * hw_ + hw_)
                for dc in range(cfg.DC):
                    nc.tensor.matmul(
                        ps[:, 0:hw_],
                        wk_sb[:, dc, pair * 128:(pair + 1) * 128],
                        x_k0[:, dc, cs],
                        start=dc == 0, stop=dc == cfg.DC - 1)
                nc.vector.tensor_copy(
                    out=khT_sb[:, pair, half * hw_:half * hw_ + hw_],
                    in_=ps[:, 0:hw_])
        for kb in range(min(4, cfg.KB_MAX)):
            Job(v_chunk(kb)).finish()

        qx = []
        qx_bufs = 2 if cfg.LEAN else 4
        for ns in range(cfg.NS if not cfg.LEAN else 1):
            x_q = xin.tile([128, cfg.DC, cfg.NSW], BF16, tag="qx",
                           bufs=qx_bufs, name="x_q")
            nc.sync.dma_start(out=x_q, in_=qT[ns])
            qx.append(x_q)
        Job(xw_slice(0, qx[0], wq_sb, qhT_sb)).finish()

        # filler jobs, keyed by the pair-0 row-range they must finish in:
        # everything keyed r is consumed by row-range r+1 (or later)
        fill_r = {0: [], 1: [], 2: []}
        if not cfg.LEAN:
            for kb in range(4, cfg.KB_MAX):
                fill_r[0].append(Job(v_chunk(kb)))
            for ns in range(1, cfg.NS_K):
                kx = xin.tile([128, cfg.DC, cfg.NSW], BF16, tag="x_ns",
                              name="kx")
                nc.sync.dma_start(out=kx, in_=kT[ns])
                fill_r[0].append(Job(xw_slice(ns, kx, wk_sb, khT_sb)))
            for ns in range(1, cfg.NS):
                fill_r[min(ns - 1, 2)].append(
                    Job(xw_slice(ns, qx[ns], wq_sb, qhT_sb)))
        else:
            for kb in range(4, cfg.KB_MAX):
                Job(v_chunk(kb)).finish()
            qk_proj(kT, wk_sb, khT_sb, ns_count=cfg.NS_K, ns_start=1)
            for ns in range(1, cfg.NS):
                x_q = xin.tile([128, cfg.DC, cfg.NSW], BF16, tag="qx",
                               bufs=qx_bufs, name="x_q")
                nc.sync.dma_start(out=x_q, in_=qT[ns])
                qx.append(x_q)
                Job(xw_slice(ns, x_q, wq_sb, qhT_sb)).finish()

        # P3 constants: emitted on the sync queue AFTER P1's input stream so
        # they don't compete for HBM bandwidth before the first matmul; they
        # transfer during P2 and are ready long before P3 needs them.
        nc.sync.dma_start(out=wo_sb, in_=woT)
        nc.sync.dma_start(out=res_sb, in_=resid)
        nc.sync.dma_start(out=g_row, in_=gamma)
        nc.sync.dma_start(out=b_row, in_=beta)
        nc.gpsimd.partition_broadcast(gamma_bc, g_row)
        nc.gpsimd.partition_broadcast(beta_bc, b_row)

        # ---- P2: attention; per-pair A2A overlaps the next pair -----------
        # Both heads' scores go into ONE 2-bank psum tile (cols h2*RNG+...)
        # so a single scalar activation computes exp for both heads.
        def ctx_mm(pair, r, kb, h2, ctx_ps, probs, nch):
            f0 = max(0, kb * 128 - r * RNG)
            h = 2 * pair + h2
            nc.tensor.matmul(
                ctx_ps[:, f0:],
                vh_sb[:, kb, h * (dh + 1):(h + 1) * (dh + 1)],
                probs[:, h2 * RNG + f0:h2 * RNG + RNG],
                start=kb == 0, stop=kb == nch - 1)

        for pair in range(cfg.PAIRS):
            for r in range(cfg.NR):
                nch = min(((r + 1) * RNG) // 128, cfg.KB_MAX)
                jobs = fill_r.pop(r, []) if pair == 0 else []
                nfill = sum(len(j.items) - j.idx for j in jobs)
                per_kb = -(-nfill // nch) if nfill else 0
                ctx_ps = [psum.tile([dh + 1, RNG], F32, tag="ctx",
                                    bufs=4, name=f"ctx_ps{h2}")
                          for h2 in range(2)]
                pend = []  # pending probs tiles awaiting their ctx matmul
                for kb in range(nch):
                    f0 = max(0, kb * 128 - r * RNG)
                    diag = f0 > 0 or kb * 128 == r * RNG
                    sc = psum.tile([128, 2 * RNG], F32, tag="sc", bufs=2,
                                   name="sc")
                    probs = att.tile([128, 2 * RNG], BF16, tag="pr",
                                     bufs=3 if cfg.LEAN else 4, name="probs")
                    # software pipeline (depth 3): ctx of kb-3 interleaves
                    # between the score matmuls of kb so the PE never waits
                    # on the tri-add + merged exp of recent chunks.
                    for h2 in range(2):
                        lo, hi = 64 * h2, 64 * h2 + 64
                        nc.tensor.matmul(
                            sc[:, h2 * RNG + f0:(h2 + 1) * RNG],
                            khT_sb[lo:hi, pair, kb * 128:(kb + 1) * 128],
                            qhT_sb[lo:hi, pair, r * RNG + f0:(r + 1) * RNG],
                            start=True, stop=True)
                        if len(pend) == 3:
                            ctx_mm(pair, r, kb - 3, h2, ctx_ps[h2],
                                   pend[0], nch)
                    if len(pend) == 3:
                        pend.pop(0)
                    if diag:
                        # causal boundary: bias the diagonal band before exp
                        for h2 in range(2):
                            band = slice(h2 * RNG + f0, h2 * RNG + f0 + 128)
                            nc.vector.tensor_add(sc[:, band], sc[:, band],
                                                 tri)
                    nc.scalar.activation(
                        out=probs, in_=sc,
                        func=mybir.ActivationFunctionType.Exp,
                        bias=pb_sb[:, kb:kb + 1],
                        scale=1.0 / math.sqrt(dh))
                    todo = per_kb
                    for j in jobs:
                        if todo <= 0 or j.done():
                            continue
                        take = min(todo, len(j.items) - j.idx)
                        j.emit(take)
                        todo -= take
                    pend.append(probs)
                for j in jobs:
                    j.finish()
                for i, pr_t in enumerate(pend):
                    for h2 in range(2):
                        ctx_mm(pair, r, nch - len(pend) + i, h2,
                               ctx_ps[h2], pr_t, nch)

                # epilogue: divide rows 0..dh-1 by row dh (the prob sum):
                # bounce the denom row to SBUF, fast-approx reciprocal
                # (the DVE RECIPROCAL op costs a flat ~3.3us!), partition
                # broadcast, then multiply straight out of PSUM.
                stage = att.tile([128, RNG], BF16, tag="stage",
                                 bufs=2 if cfg.LEAN else 4)
                sb = 1 if cfg.LEAN else 2
                for h2 in range(2):
                    den = small.tile([1, RNG], F32, tag=f"den{h2}",
                                     name=f"den{h2}", bufs=sb)
                    nc.vector.tensor_copy(out=den, in_=ctx_ps[h2][dh:dh + 1])
                    rec = small.tile([1, RNG], F32, tag=f"rec{h2}",
                                     name=f"rec{h2}", bufs=sb)
                    nc.vector.reciprocal_approx_fast(rec, den)
                    rbc = small.tile([64, RNG], F32, tag=f"rbc{h2}",
                                     name=f"rbc{h2}", bufs=sb)
                    nc.gpsimd.partition_broadcast(rbc, rec)
                    nc.vector.tensor_mul(
                        stage[64 * h2:64 * h2 + 64, :],
                        ctx_ps[h2][0:dh, :], rbc)
                # stage rows r*RNG+[0,RNG) as two A2A slots of RSL rows
                for j in range(2):
                    nc.sync.dma_start(
                        out=a2a_in[pair][2 * r + j, :, :],
                        in_=stage[:, j * RSL:(j + 1) * RSL])
            nc.gpsimd.collective_compute(
                "AllToAll", mybir.AluOpType.bypass,
                replica_groups=[list(range(cfg.NC))],
                ins=[a2a_in[pair][:]], outs=[a2a_out[pair][:]])

        # Scheduler fence (no runtime syncs): without it the scheduler hoists
        # the cc0-gated fetch DMAs ahead of pair-1's staging DMAs on the sync
        # queue, stalling the whole queue until cc0 completes.
        tc.no_sync_barrier()

        # fetch gathered ctx chunks: ccb[(pair, sender)] = sender's 2 heads
        # (128 dims) of pair `pair`, for my RQ rows (RSL per batch half).
        ccb = {}
        for pair in range(cfg.PAIRS):
            for s in range(cfg.NC):
                t_ccb = ctxf.tile([128, RSL], BF16, name=f"ccb_{pair}_{s}",
                                  tag=f"ccb_{pair}_{s}")
                nc.sync.dma_start(out=t_ccb, in_=a2a_out[pair][s, :, :])
                ccb[(pair, s)] = t_ccb

        # ---- P3: Wo + residual + LayerNorm ---------------------------------
        # row-tile t covers my rows [t*128,(t+1)*128): batch b = t//2,
        # in-slot column range (t%2)*128. Sender 4b+s holds head chunk
        # (pair, s) for that batch. Round A accumulates every pair-0 chunk
        # for ALL row-tiles into SBUF partials while the pair-1 collective
        # is still in flight; round B adds the pair-1 chunks.
        def wo_round(t, pair, pso):
            b = t // 2
            col = slice((t % 2) * 128, (t % 2) * 128 + 128)
            for s in range(cfg.G):
                cc = ccb[(pair, 4 * b + s)][:, col]
                # global output dim chunk for (sender s, pair):
                oc = s * cfg.D4C + pair
                for nsl in range(cfg.WON):
                    nc.tensor.matmul(
                        pso[nsl], cc,
                        wo_sb[:, oc, nsl * cfg.WONW:(nsl + 1) * cfg.WONW],
                        start=s == 0, stop=s == cfg.G - 1)

        partA = []
        for t in range(RQ // 128):
            pso = [psum.tile([128, cfg.WONW], F32, tag="ctx",
                             bufs=4, name=f"psoA{nsl}")
                   for nsl in range(cfg.WON)]
            wo_round(t, 0, pso)
            pa = lnp.tile([128, D], BF16, tag="partA", bufs=4)
            for nsl in range(cfg.WON):
                sl = slice(nsl * cfg.WONW, (nsl + 1) * cfg.WONW)
                nc.vector.tensor_add(pa[:, sl], pso[nsl], res_sb[:, t, sl])
            partA.append(pa)

        for t in range(RQ // 128):
            pso = [psum.tile([128, cfg.WONW], F32, tag="ctx",
                             bufs=4, name=f"psoB{nsl}")
                   for nsl in range(cfg.WON)]
            wo_round(t, 1, pso)
            x = lnp.tile([128, D], F32, tag="x")
            for nsl in range(cfg.WON):
                sl = slice(nsl * cfg.WONW, (nsl + 1) * cfg.WONW)
                nc.vector.tensor_add(x[:, sl], pso[nsl], partA[t][:, sl])
            fmax = math.gcd(nc.vector.BN_STATS_FMAX, D)
            nsub = D // fmax
            stats = lnp.tile([128, nsub, nc.vector.BN_STATS_DIM], F32,
                             tag="stats")
            for sg in range(nsub):
                nc.vector.bn_stats(
                    out=stats[:, sg, :],
                    in_=x.rearrange("p (a b) -> p a b", a=nsub)[:, sg, :])
            mv = lnp.tile([128, nc.vector.BN_AGGR_DIM], F32, tag="mv")
            nc.vector.bn_aggr(out=mv, in_=stats)
            sd = lnp.tile([128, 1], F32, tag="sd")
            nc.scalar.activation(out=sd, in_=mv[:, 1:2],
                                 func=mybir.ActivationFunctionType.Sqrt,
                                 bias=eps_sb, scale=1.0)
            rstd = lnp.tile([128, 1], F32, tag="rstd")
            nc.vector.reciprocal_approx_fast(rstd, sd)
            y = lnp.tile([128, D], BF16, tag="y")
            nc.vector.tensor_scalar(
                out=y, in0=x, scalar1=mv[:, 0:1], scalar2=rstd,
                op0=mybir.AluOpType.subtract, op1=mybir.AluOpType.mult)
            yg = lnp.tile([128, D], BF16, tag="yg")
            nc.vector.tensor_mul(yg, y, gamma_bc)
            out_sb = lnp.tile([128, D], BF16, tag="out_sb")
            nc.vector.tensor_add(out_sb, yg, beta_bc)
            nc.sync.dma_start(out=out_shard[t * 128:(t + 1) * 128, :],
                              in_=out_sb)

    nc.compile()
    return nc


def _tile_x(xT, ns_count, nsw, dc=8):
    """[D, S'] -> [ns, 128, dc, nsw] so each n-slice DMA is contiguous."""
    d, s = xT.shape
    cols = ns_count * nsw
    out = xT[:, :cols].reshape(dc, 128, ns_count, nsw)
    return np.ascontiguousarray(out.transpose(2, 1, 0, 3))


def _tile_w(wT):
    """[D, O] -> [128, dc, O] so the weight DMA is contiguous."""
    d, o = wT.shape
    return np.ascontiguousarray(wT.reshape(d // 128, 128, o).transpose(1, 0, 2))


def make_in_maps(cfg: Cfg, q, k, v, Wq, Wk, Wv, Wo, gamma, beta, sen_len):
    """Host-side sharding: slice/transpose/cast/tile per core."""
    bf = ml_dtypes.bfloat16
    in_maps = []
    woT_full = _tile_w(Wo.T.astype(bf))
    pos = np.arange(cfg.S)
    per_batch = {}
    for b in range(cfg.B):
        per_batch[b] = (
            _tile_x(q[b].T.astype(bf), cfg.NS, cfg.NSW),
            _tile_x(k[b].T.astype(bf), cfg.NS_K, cfg.NSW),
            _tile_x(v[b].T.astype(bf), cfg.KB_MAX, 128),
            np.ascontiguousarray(
                np.where(pos < int(sen_len[b]), 0.0, NEG_INF)
                .astype(np.float32).reshape(cfg.KCH, 128).T),
        )
    for c in range(cfg.NC):
        g = c // cfg.G
        l = c % cfg.G
        hs = slice(l * cfg.D4, (l + 1) * cfg.D4)
        rows = slice(c * cfg.RSL, (c + 1) * cfg.RSL)
        qTb, kTb, vTb, pb = per_batch[g]
        res = np.concatenate([q[b, rows, :] for b in range(cfg.B)], axis=0)
        res = res.astype(np.float32).reshape(cfg.G, 128, cfg.D)
        in_maps.append({
            "qT": qTb, "kT": kTb, "vT": vTb,
            "wqT": _tile_w(Wq[hs, :].T.astype(bf)),
            "wkT": _tile_w(Wk[hs, :].T.astype(bf)),
            "wvT": _tile_w(Wv[hs, :].T.astype(bf)),
            "woT": woT_full,
            "resid": np.ascontiguousarray(res.transpose(1, 0, 2)),
            "pad_bias": pb,
            "gamma": gamma.reshape(1, cfg.D).astype(bf),
            "beta": beta.reshape(1, cfg.D).astype(bf),
        })
    return in_maps


def assemble_output(cfg: Cfg, results):
    out = np.empty((cfg.B, cfg.S, cfg.D), np.float32)
    for c in range(cfg.NC):
        rows = slice(c * cfg.RSL, (c + 1) * cfg.RSL)
        for b in range(cfg.B):
            out[b, rows, :] = results[c]["out_shard"][
                b * cfg.RSL:(b + 1) * cfg.RSL].astype(np.float32)
    return out


_PROGRAM_CACHE = {}


def _get_program(cfg: Cfg):
    key = (cfg.B, cfg.S, cfg.D, cfg.H, cfg.dh, cfg.KB_MAX)
    if key not in _PROGRAM_CACHE:
        _PROGRAM_CACHE[key] = build_program(cfg)
    return _PROGRAM_CACHE[key]


def run(cfg: Cfg, inputs: dict, trace: bool = False):
    nc = _get_program(cfg)
    in_maps = make_in_maps(cfg, **inputs)
    res = run_bass_kernel_spmd(nc, in_maps, core_ids=list(range(cfg.NC)),
                               trace=trace)
    return assemble_output(cfg, res.results), res


def kernel(**inputs) -> np.ndarray:
    kmax = int(np.max(inputs["sen_len"]))
    cfg = Cfg(B=2, S=2048, D=1024, H=16, dh=64, kmax=kmax)
    out, _ = run(cfg, inputs)
    return out
